# revision 22
# baseline (speedup 1.0000x reference)
"""Trainium2 Bass kernel for causal GQA attention block (dense transformer).

Full module: qkv = clip(x @ Wqkv.T, +-8); half-split RoPE on q,k;
GQA causal attention (32 q heads, 8 kv heads, head_dim 128); out @ Wout.T.

Sharding: tensor-parallel over heads across 8 cores. Each core owns 4 q
heads + their shared kv head (rows of Wqkv) and the matching 512 columns
of Wout; it computes a full-shape partial of the output projection and the
host sums the 8 partials.

All matmuls run in fp16 (10-bit mantissa ~ tf32 precision for this data,
1 cycle/row on the PE with fast weight load). Softmax skips the max
subtraction (scores are O(5) with this data; softmax is shift invariant;
exp fits fp16 range) so the denominator comes from a ones-vector matmul;
normalization multiplies the PSUM eviction of the attention output by a
GpSimd partition-broadcast of the reciprocal.

PSUM banks are partitioned so attention is never starved while the qkv
projection of the next chunk accumulates: the qkv GEMM runs in 3 passes
of 2 r-tiles over x-tiles resident in SBUF, holding only 2 banks (PQ);
scores (PS, 2), pv+output-projection (PVF, 3), and the softmax sums
(PM, 1) own the rest.
"""
import os
import sys
import math

for _p in ("/opt/trn_rl_repo", "/root/.axon_site/_ro/trn_rl_repo"):
    if os.path.isdir(_p) and _p not in sys.path:
        sys.path.insert(0, _p)

import numpy as np

import concourse.bass as bass
import concourse.tile as tile
from concourse import bacc, mybir
from concourse import bass_utils

# If BASS_TRACE is set in the environment, run_bass_kernel_spmd imports
# antenv.axon_hooks, which this image's antenv package lacks. Register a
# stub so tracing degrades gracefully instead of crashing.
try:
    import antenv.axon_hooks  # noqa: F401
except ImportError:
    try:
        import types
        import antenv

        _hooks = types.ModuleType("antenv.axon_hooks")
        _hooks._hook = None
        _hooks.set_axon_ntff_profile_hook = (
            lambda h: setattr(_hooks, "_hook", h))
        _hooks.get_axon_ntff_profile_hook = lambda: _hooks._hook
        sys.modules["antenv.axon_hooks"] = _hooks
        antenv.axon_hooks = _hooks
    except Exception:  # noqa: BLE001
        pass

F32 = mybir.dt.float32
F16 = mybir.dt.float16
AF = mybir.ActivationFunctionType
OP = mybir.AluOpType

NEG_BIG = -1.0e30


def default_cfg():
    return dict(
        B=2, L=2048, D=4096, QH=4, HD=128,
        CH=512,   # attention q-chunk width
        QN=512,   # qkv projection l-chunk width
        OG=512,   # output projection column-group width
        CLIP=8.0, theta=500000.0, ncores=8,
    )


def mini_cfg():
    return dict(
        B=2, L=512, D=512, QH=2, HD=128,
        CH=256, QN=256, OG=256,
        CLIP=8.0, theta=500000.0, ncores=1,
    )


def build_program(cfg):
    B, L, D = cfg["B"], cfg["L"], cfg["D"]
    QH, HD = cfg["QH"], cfg["HD"]
    CH, QN, OG = cfg["CH"], cfg["QN"], cfg["OG"]
    CLIP = cfg["CLIP"]
    RT = QH + 2               # r-tiles per core: QH q heads, k, v
    R = RT * HD
    CT = D // HD              # contraction tiles
    TPC = CH // HD            # k-tiles per attention chunk
    NJ = L // CH              # attention q-chunks per batch
    NLC = L // QN
    scale = 1.0 / math.sqrt(HD)

    nc = bacc.Bacc("TRN2", target_bir_lowering=False, debug=False,
                   enable_asserts=False, num_devices=1)

    CT2 = CT // 2
    # x and wq are staged host-side in ci-pair-major layout so each DMA
    # start moves a 256KB+ contiguous block (one engine stream each;
    # issue rate on the sync engine is the DMA concurrency limit)
    xH_d = nc.dram_tensor("xH", [B * NLC * CT2 * HD, 2 * QN], F16,
                          kind="ExternalInput").ap()
    wq_d = nc.dram_tensor("wq", [CT2 * HD, 2 * R], F16,
                          kind="ExternalInput").ap()
    wo_d = nc.dram_tensor("wo", [QH * HD, D], F16, kind="ExternalInput").ap()
    ra_d = nc.dram_tensor("ropeA", [HD, L], F16, kind="ExternalInput").ap()
    rb_d = nc.dram_tensor("ropeB", [HD, L], F32, kind="ExternalInput").ap()
    pm_d = nc.dram_tensor("perm", [HD, HD], F16, kind="ExternalInput").ap()
    ms_d = nc.dram_tensor("mask", [HD, HD], F32, kind="ExternalInput").ap()
    ok_d = nc.dram_tensor("onesk", [HD, 1], F16, kind="ExternalInput").ap()
    id_d = nc.dram_tensor("ident", [HD, HD], F16, kind="ExternalInput").ap()
    out_d = nc.dram_tensor("out", [B * L, D], F16, kind="ExternalOutput").ap()

    from contextlib import ExitStack
    with tile.TileContext(nc) as tc, ExitStack() as _es:
        wq_pool = _es.enter_context(tc.tile_pool(name="wq_pool", bufs=1))
        wo_pool = _es.enter_context(tc.tile_pool(name="wo_pool", bufs=1))
        cpool = _es.enter_context(tc.tile_pool(name="const", bufs=1))
        ex_pool = _es.enter_context(tc.tile_pool(name="ex", bufs=3))
        at_pool = _es.enter_context(tc.tile_pool(name="at", bufs=2 * QH + 1))
        rc_pool = _es.enter_context(tc.tile_pool(name="rc", bufs=2))
        bc_pool = _es.enter_context(tc.tile_pool(name="bc", bufs=2))
        fo_pool = _es.enter_context(tc.tile_pool(name="fo", bufs=3))
        qkv_pool = _es.enter_context(tc.tile_pool(name="qkv", bufs=2))
        qp_pool = _es.enter_context(tc.tile_pool(name="qp", bufs=1))
        x_pool = _es.enter_context(tc.tile_pool(name="xp", bufs=20))
        cl_pool = _es.enter_context(tc.tile_pool(name="cl", bufs=1))
        rt_pool = _es.enter_context(tc.tile_pool(name="rt", bufs=2))
        tb_pool = _es.enter_context(tc.tile_pool(name="tb", bufs=2))
        PS = _es.enter_context(tc.tile_pool(name="PS", bufs=2, space="PSUM"))
        PVF = _es.enter_context(tc.tile_pool(name="PVF", bufs=3,
                                             space="PSUM"))
        PM = _es.enter_context(tc.tile_pool(name="PM", bufs=1, space="PSUM"))
        PQ = _es.enter_context(tc.tile_pool(name="PQ", bufs=2, space="PSUM"))
        t_perm = cpool.tile([HD, HD], F16, tag="t_perm")
        t_mask = cpool.tile([HD, HD], F32, tag="t_mask")
        t_onesk = cpool.tile([HD, 1], F16, tag="t_onesk")
        t_ident = cpool.tile([HD, HD], F16, tag="t_ident")
        nc.sync.dma_start(t_perm[:], pm_d[:])
        nc.sync.dma_start(t_mask[:], ms_d[:])
        nc.sync.dma_start(t_onesk[:], ok_d[:])
        nc.sync.dma_start(t_ident[:], id_d[:])

        def emit_x_dmas(b, lc):
            xts = []
            for g in range(CT2):
                xt = x_pool.tile([HD, 2 * QN], F16, tag="xt",
                                 name=f"xt{b}_{lc}_{g}")
                row0 = ((b * NLC + lc) * CT2 + g) * HD
                nc.sync.dma_start(xt[:], xH_d[row0:row0 + HD, :])
                xts.append(xt)
            return xts

        def x_sl(xts, ci):
            return xts[ci // 2][:, (ci % 2) * QN:(ci % 2 + 1) * QN]

        def w_sl(ci, r):
            off = (ci % 2) * R + r * HD
            return w_ci[ci // 2][:, off:off + HD]

        # k,v first so attention on a chunk unblocks one pass earlier
        groups = [(QH, QH + 1)] + [(2 * g, 2 * g + 1) for g in range(QH // 2)]

        # first x chunk interleaved with the qkv weights so the first
        # pass streams both as they land; leading weight tiles split in
        # halves so the first matmuls see their data sooner
        first_xts = []
        w_ci = [wq_pool.tile([HD, 2 * R], F16, tag=f"w{g}", name=f"w{g}")
                for g in range(CT2)]
        for g in range(CT2):
            xt = x_pool.tile([HD, 2 * QN], F16, tag="xt",
                             name=f"xt0_0_{g}")
            nc.sync.dma_start(xt[:], xH_d[g * HD:(g + 1) * HD, :])
            first_xts.append(xt)
            if g < 4:
                nc.sync.dma_start(w_ci[g][:, :R], wq_d[g * HD:(g + 1) * HD, :R])
                nc.sync.dma_start(w_ci[g][:, R:], wq_d[g * HD:(g + 1) * HD, R:])
            else:
                nc.sync.dma_start(w_ci[g][:], wq_d[g * HD:(g + 1) * HD, :])
        # resident output-projection weights (DMAs emitted after the
        # first chunk; only needed ~100us in, at the first emit_fin)
        wo_sb = wo_pool.tile([HD, QH * D], F16, tag="wo_sb", name="wo_sb")

        # q/k/v tiles shared across batches; q is consumed within its own
        # chunk so one slot per tag suffices, k/v need two (cross-batch)
        q_t = [[[qp_pool.tile([HD, QN], F16, tag=f"q{h}_{lc}",
                              name=f"q{h}_{b}_{lc}")
                 for lc in range(NLC)]
                for h in range(QH)]
               for b in range(B)]
        k_t = [[qkv_pool.tile([HD, QN], F16, tag=f"k_{lc}",
                              name=f"k_{b}_{lc}")
                for lc in range(NLC)]
               for b in range(B)]
        v_t = [[qkv_pool.tile([HD, QN], F16, tag=f"v_{lc}",
                              name=f"v_{b}_{lc}")
                for lc in range(NLC)]
               for b in range(B)]

        def emit_rope(b, lc, r, cl, t_ra, t_rb):
            """RoPE for q head r (r<QH) or k (r==QH) from clipped tile."""
            dest = (q_t[b][r][lc] if r < QH else k_t[b][lc])[:]
            rot = PQ.tile([HD, QN], F32, tag="p",
                          name=f"rot{b}_{lc}_{r}")
            nc.tensor.matmul(rot[:], t_perm[:], cl[:],
                             start=True, stop=True)
            t1 = rt_pool.tile([HD, QN], F16, tag="t1")
            nc.vector.tensor_tensor(t1[:], rot[:], t_rb[:], OP.mult)
            nc.vector.tensor_tensor(dest, cl[:], t_ra[:], OP.mult)
            nc.vector.tensor_tensor(dest, dest, t1[:], OP.add)

        def emit_qkv_chunk(b, lc, xts=None):
            t_ra = tb_pool.tile([HD, QN], F16, tag="t_ra",
                                name=f"t_ra{b}_{lc}")
            t_rb = tb_pool.tile([HD, QN], F32, tag="t_rb",
                                name=f"t_rb{b}_{lc}")
            lsl0 = slice(lc * QN, (lc + 1) * QN)
            nc.sync.dma_start(t_ra[:], ra_d[:, lsl0])
            nc.sync.dma_start(t_rb[:], rb_d[:, lsl0])
            if xts is None:
                xts = emit_x_dmas(b, lc)
            # 3 passes of 2 r-tiles over the resident x tiles: holds only
            # the 2 PQ banks so attention keeps its own banks meanwhile.
            for r0, r1 in groups:
                acc = [PQ.tile([HD, QN], F32, tag="p",
                               name=f"acc{b}_{lc}_{r}")
                       for r in (r0, r1)]
                for ci in range(CT):
                    for i, r in enumerate((r0, r1)):
                        nc.tensor.matmul(
                            acc[i][:],
                            w_sl(ci, r),
                            x_sl(xts, ci),
                            start=(ci == 0), stop=(ci == CT - 1))
                for i, r in enumerate((r0, r1)):
                    cl = cl_pool.tile([HD, QN], F16, tag=f"cl{r}",
                                      name=f"cl{b}_{lc}_{r}")
                    nc.vector.tensor_scalar(
                        cl[:], acc[i][:], -CLIP, CLIP, OP.max, OP.min)
                    if r < QH + 1:
                        emit_rope(b, lc, r, cl, t_ra, t_rb)
                    else:
                        vtr = PQ.tile([HD, QN], F16, tag="p",
                                      name=f"vtr{b}_{lc}")
                        nt = QN // HD
                        for t in range(nt):
                            nc.tensor.matmul(
                                vtr[:, t * HD:(t + 1) * HD],
                                cl[:, t * HD:(t + 1) * HD],
                                t_ident[:],
                                is_transpose=True,
                                start=(t == 0), stop=(t == nt - 1))
                        nc.vector.tensor_copy(v_t[b][lc][:], vtr[:])

        def emit_attn_chunk(b, j):
            at_tiles = []
            for h in range(QH):
                pv = PVF.tile([HD, CH], F32, tag="p",
                              name=f"pv{b}_{j}_{h}")
                sm = PM.tile([1, CH], F32, tag="p",
                             name=f"sm{b}_{j}_{h}")
                nk = (j + 1) * TPC
                for ki in range(nk):
                    diag = ki >= j * TPC
                    w0 = (ki - j * TPC) * HD if diag else 0
                    W = CH - w0
                    klc, kof = divmod(ki * HD, QN)
                    sc = PS.tile([HD, CH], F32, tag="p",
                                 name=f"sc{b}_{j}_{h}_{ki}")
                    nc.tensor.matmul(
                        sc[:, :W],
                        k_t[b][klc][:, kof:kof + HD],
                        q_t[b][h][j][:, w0:w0 + W],
                        start=True, stop=True)
                    ex = ex_pool.tile([HD, CH], F16, tag="ex")
                    if diag:
                        nc.vector.tensor_tensor(
                            sc[:, :HD], sc[:, :HD], t_mask[:], OP.add)
                    nc.scalar.activation(
                        ex[:, :W], sc[:, :W], AF.Exp, scale=scale)
                    nc.tensor.matmul(
                        pv[:, w0:w0 + W],
                        v_t[b][klc][:, kof:kof + HD],
                        ex[:, :W],
                        start=(ki == 0), stop=(ki == nk - 1))
                    nc.tensor.matmul(
                        sm[0:1, w0:w0 + W],
                        t_onesk[:], ex[:, :W],
                        start=(ki == 0), stop=(ki == nk - 1))
                rc32 = rc_pool.tile([1, CH], F32, tag="rc32",
                                    name=f"rc32_{b}_{j}_{h}")
                nc.vector.reciprocal_approx_fast(rc32[:], sm[:])
                bcs = bc_pool.tile([HD, CH], F32, tag="bcs",
                                   name=f"bcs_{b}_{j}_{h}")
                nc.gpsimd.partition_broadcast(bcs[:], rc32[:])
                at = at_pool.tile([HD, CH], F16, tag="at",
                                  name=f"at{b}_{j}_{h}")
                nc.vector.tensor_tensor(at[:], pv[:], bcs[:], OP.mult)
                at_tiles.append(at)
            return at_tiles

        def emit_fin(b, j, at_tiles, tail=False):
            for lt in range(TPC):
                fo = None
                for oc in range(D // OG):
                    fn = PVF.tile([HD, OG], F32, tag="p",
                                  name=f"fn{b}_{j}_{lt}_{oc}")
                    for i in range(QH):
                        nc.tensor.matmul(
                            fn[:],
                            at_tiles[i][:, lt * HD:(lt + 1) * HD],
                            wo_sb[:, i * D + oc * OG:
                                  i * D + (oc + 1) * OG],
                            start=(i == 0), stop=(i == QH - 1))
                    row0 = b * L + j * CH + lt * HD
                    if tail:
                        # smaller parallel stores drain the kernel tail
                        # faster than one long per-engine stream
                        fo = fo_pool.tile([HD, 2 * OG], F16, tag="fo")
                        if oc % 2 == 0:
                            nc.vector.tensor_copy(fo[:, :OG], fn[:])
                        else:
                            nc.scalar.copy(fo[:, :OG], fn[:])
                        nc.sync.dma_start(
                            out_d[row0:row0 + HD,
                                  oc * OG:(oc + 1) * OG],
                            fo[:, :OG])
                        continue
                    # pairs of column groups share one fo tile so the
                    # out DMA moves 256KB per start
                    half = oc % 2
                    if half == 0:
                        fo = fo_pool.tile([HD, 2 * OG], F16, tag="fo")
                        nc.vector.tensor_copy(fo[:, :OG], fn[:])
                    else:
                        nc.scalar.copy(fo[:, OG:], fn[:])
                        nc.sync.dma_start(
                            out_d[row0:row0 + HD,
                                  (oc - 1) * OG:(oc + 1) * OG],
                            fo[:])

        prev = None
        for b in range(B):
            for lc in range(NLC):
                first = (b == 0 and lc == 0)
                emit_qkv_chunk(b, lc, xts=first_xts if first else None)
                at_tiles = emit_attn_chunk(b, j=lc)
                if b == 0 and lc == 1:
                    # after chunk 1's x DMAs so they aren't queued
                    # behind these 4 MB; first use is ~one chunk later
                    for i in range(QH):
                        nc.sync.dma_start(wo_sb[:, i * D:(i + 1) * D],
                                          wo_d[i * HD:(i + 1) * HD, :])
                if prev is not None:
                    emit_fin(prev[0], prev[1], prev[2])
                prev = (b, lc, at_tiles)
        emit_fin(prev[0], prev[1], prev[2], tail=True)
    nc.compile()
    return nc


def host_tables(cfg):
    L, HD, theta = cfg["L"], cfg["HD"], cfg["theta"]
    half = HD // 2
    inv_freq = 1.0 / (theta ** (np.arange(half, dtype=np.float64) / half))
    ang = np.arange(L, dtype=np.float64)[:, None] * inv_freq[None, :]  # [L,half]
    cos = np.cos(ang).astype(np.float32)   # [L, half]
    sin = np.sin(ang).astype(np.float32)
    ropeA = np.empty((HD, L), dtype=np.float32)
    ropeB = np.empty((HD, L), dtype=np.float32)
    ropeA[:half] = cos.T
    ropeA[half:] = cos.T
    ropeB[:half] = sin.T
    ropeB[half:] = sin.T

    perm = np.zeros((HD, HD), dtype=np.float32)
    for d in range(half):
        perm[d + half, d] = -1.0          # rot[d<64] = -q[d+64]
    for d in range(half, HD):
        perm[d - half, d] = 1.0           # rot[d>=64] = +q[d-64]

    mask = np.where(np.arange(HD)[None, :] >= np.arange(HD)[:, None],
                    0.0, NEG_BIG).astype(np.float32)  # [k, q]
    onesk = np.ones((HD, 1), dtype=np.float32)
    ident = np.eye(HD, dtype=np.float32)
    return dict(ropeA=ropeA.astype(np.float16), ropeB=ropeB,
                perm=perm.astype(np.float16), mask=mask,
                onesk=onesk.astype(np.float16),
                ident=ident.astype(np.float16))


def host_x_layout(cfg, x):
    """[B,L,D] -> ci-pair-major rows [(b,lc,g,p), (u,t)] in fp16."""
    B, L, D, HD = cfg["B"], cfg["L"], cfg["D"], cfg["HD"]
    QN = cfg["QN"]
    NLC = L // QN
    CT2 = D // HD // 2
    arr = x.reshape(B, NLC, QN, CT2, 2, HD).transpose(0, 1, 3, 5, 4, 2)
    return np.ascontiguousarray(
        arr.reshape(B * NLC * CT2 * HD, 2 * QN).astype(np.float16))


def host_wq_layout(cfg, wqT):
    """[D,R] -> ci-pair-major rows [(g,p), (u,rcol)] in fp16."""
    D, HD = cfg["D"], cfg["HD"]
    R = wqT.shape[1]
    CT2 = D // HD // 2
    arr = wqT.reshape(CT2, 2, HD, R).transpose(0, 2, 1, 3)
    return np.ascontiguousarray(
        arr.reshape(CT2 * HD, 2 * R).astype(np.float16))


def host_in_maps(cfg, x, Wqkv, Wout):
    """Build per-core input maps from the full tensors."""
    B, L, D, QH, HD = cfg["B"], cfg["L"], cfg["D"], cfg["QH"], cfg["HD"]
    nco = cfg["ncores"]
    tabs = host_tables(cfg)
    xH = host_x_layout(cfg, np.asarray(x))
    NHT = QH * nco      # total q heads
    in_maps = []
    for c in range(nco):
        q_rows = np.arange(c * QH * HD, (c + 1) * QH * HD)
        k_rows = np.arange(NHT * HD + c * HD, NHT * HD + (c + 1) * HD)
        v_rows = np.arange(NHT * HD + nco * HD + c * HD,
                           NHT * HD + nco * HD + (c + 1) * HD)
        rows = np.concatenate([q_rows, k_rows, v_rows])
        wq = host_wq_layout(cfg, Wqkv[rows, :].T)
        cols = np.arange(c * QH * HD, (c + 1) * QH * HD)
        wo = np.ascontiguousarray(Wout[:, cols].T.astype(np.float16))
        m = dict(xH=xH, wq=wq, wo=wo)
        m.update(tabs)
        in_maps.append(m)
    return in_maps


_PROGRAM_CACHE = {}
LAST_RESULTS = None


def _get_program(cfg_key, cfg):
    if cfg_key not in _PROGRAM_CACHE:
        _PROGRAM_CACHE[cfg_key] = build_program(cfg)
    return _PROGRAM_CACHE[cfg_key]


def kernel(x, Wqkv, Wout):
    cfg = default_cfg()
    B, L, D = cfg["B"], cfg["L"], cfg["D"]
    x = np.asarray(x, dtype=np.float32)
    Wqkv = np.asarray(Wqkv, dtype=np.float32)
    Wout = np.asarray(Wout, dtype=np.float32)
    nc = _get_program("full", cfg)
    in_maps = host_in_maps(cfg, x, Wqkv, Wout)
    res = bass_utils.run_bass_kernel_spmd(
        nc, in_maps, core_ids=list(range(cfg["ncores"])))
    global LAST_RESULTS
    LAST_RESULTS = res
    acc = np.zeros((B * L, D), dtype=np.float32)
    for c in range(cfg["ncores"]):
        acc += res.results[c]["out"].astype(np.float32)
    return acc.reshape(B, L, D)


# ---------------------------------------------------------------------------
# dev helpers (not used by the grading harness)

def _np_partial_reference(cfg, x, Wqkv_rows, Wout_cols_T):
    """Numpy reference for ONE core's partial output.

    Wqkv_rows: [R, D] (q heads, k, v rows for this core)
    Wout_cols_T: [QH*HD, D] (transposed slice of Wout columns)
    """
    B, L, D, QH, HD = cfg["B"], cfg["L"], cfg["D"], cfg["QH"], cfg["HD"]
    CLIP, theta = cfg["CLIP"], cfg["theta"]
    half = HD // 2
    xf = x.reshape(B * L, D).astype(np.float64)
    qkv = np.clip(xf @ Wqkv_rows.astype(np.float64).T, -CLIP, CLIP)
    qkv = qkv.reshape(B, L, (QH + 2), HD)
    q = qkv[:, :, :QH, :]            # [B, L, QH, HD]
    k = qkv[:, :, QH, :]             # [B, L, HD]
    v = qkv[:, :, QH + 1, :]         # [B, L, HD]

    inv_freq = 1.0 / (theta ** (np.arange(half, dtype=np.float64) / half))
    ang = np.arange(L, dtype=np.float64)[:, None] * inv_freq[None, :]
    cos, sin = np.cos(ang), np.sin(ang)      # [L, half]

    def rope(t):  # t [B, L, ..., HD] with positions on axis 1
        t1, t2 = t[..., :half], t[..., half:]
        shape = [1, L] + [1] * (t.ndim - 3) + [half]
        c = cos.reshape(L, half).reshape(shape)
        s = sin.reshape(L, half).reshape(shape)
        return np.concatenate([t1 * c - t2 * s, t2 * c + t1 * s], axis=-1)

    q = rope(q)
    k = rope(k)
    scalev = 1.0 / math.sqrt(HD)
    causal = np.tril(np.ones((L, L), dtype=bool))
    outs = []
    for bi in range(B):
        heads = []
        for h in range(QH):
            s = (q[bi, :, h, :] @ k[bi].T) * scalev
            s = np.where(causal, s, -np.inf)
            p = np.exp(s - s.max(axis=-1, keepdims=True))
            p /= p.sum(axis=-1, keepdims=True)
            heads.append(p @ v[bi])
        attn = np.concatenate(heads, axis=-1)     # [L, QH*HD]
        outs.append(attn)
    attn = np.stack(outs, 0).reshape(B * L, QH * HD)
    return (attn @ Wout_cols_T.astype(np.float64)).astype(np.float32)


def _mini_test(mode="sim"):
    from concourse.bass_interp import CoreSim
    cfg = mini_cfg()
    B, L, D, QH, HD = cfg["B"], cfg["L"], cfg["D"], cfg["QH"], cfg["HD"]
    R = (QH + 2) * HD
    rng = np.random.default_rng(0)
    x = (rng.standard_normal((B, L, D)) * 1.0).astype(np.float32)
    Wqkv_rows = (rng.standard_normal((R, D)) * D ** -0.5).astype(np.float32)
    WoT = (rng.standard_normal((QH * HD, D)) * D ** -0.5).astype(np.float32)

    nc = build_program(cfg)
    tabs = host_tables(cfg)
    xH = host_x_layout(cfg, x)
    wq = host_wq_layout(cfg, Wqkv_rows.T.astype(np.float64))
    in_map = dict(xH=xH, wq=wq, wo=WoT.astype(np.float16))
    in_map.update(tabs)

    want = _np_partial_reference(cfg, x, Wqkv_rows, WoT)

    if mode == "sim":
        sim = CoreSim(nc, trace=False)
        for kk, vv in in_map.items():
            sim.tensor(kk)[:] = vv
        sim.simulate(check_with_hw=False)
        got = np.array(sim.tensor("out")).astype(np.float32)
    else:
        res = bass_utils.run_bass_kernel_spmd(nc, [in_map], core_ids=[0])
        got = res.results[0]["out"].astype(np.float32)
    relmax = np.abs(got - want).max() / np.abs(want).max()
    rel2 = np.linalg.norm(got - want) / np.linalg.norm(want)
    print(f"mini {mode}: relmax={relmax:.3e} rel2={rel2:.3e}")


if __name__ == "__main__":
    _mini_test(sys.argv[1] if len(sys.argv) > 1 else "sim")


# revision 25
# speedup vs baseline: 1.0077x; 1.0077x over previous
"""Trainium2 Bass kernel for causal GQA attention block (dense transformer).

Full module: qkv = clip(x @ Wqkv.T, +-8); half-split RoPE on q,k;
GQA causal attention (32 q heads, 8 kv heads, head_dim 128); out @ Wout.T.

Sharding: tensor-parallel over heads across 8 cores. Each core owns 4 q
heads + their shared kv head (rows of Wqkv) and the matching 512 columns
of Wout; it computes a full-shape partial of the output projection and the
host sums the 8 partials.

All matmuls run in fp16 (10-bit mantissa ~ tf32 precision for this data,
1 cycle/row on the PE with fast weight load). Softmax skips the max
subtraction (scores are O(5) with this data; softmax is shift invariant;
exp fits fp16 range) so the denominator comes from a ones-vector matmul;
normalization multiplies the PSUM eviction of the attention output by a
GpSimd partition-broadcast of the reciprocal.

PSUM banks are partitioned so attention is never starved while the qkv
projection of the next chunk accumulates: the qkv GEMM runs in 3 passes
of 2 r-tiles over x-tiles resident in SBUF, holding only 2 banks (PQ);
scores (PS, 2), pv+output-projection (PVF, 3), and the softmax sums
(PM, 1) own the rest.
"""
import os
import sys
import math

for _p in ("/opt/trn_rl_repo", "/root/.axon_site/_ro/trn_rl_repo"):
    if os.path.isdir(_p) and _p not in sys.path:
        sys.path.insert(0, _p)

import numpy as np

import concourse.bass as bass
import concourse.tile as tile
from concourse import bacc, mybir
from concourse import bass_utils

# If BASS_TRACE is set in the environment, run_bass_kernel_spmd imports
# antenv.axon_hooks, which this image's antenv package lacks. Register a
# stub so tracing degrades gracefully instead of crashing.
try:
    import antenv.axon_hooks  # noqa: F401
except ImportError:
    try:
        import types
        import antenv

        _hooks = types.ModuleType("antenv.axon_hooks")
        _hooks._hook = None
        _hooks.set_axon_ntff_profile_hook = (
            lambda h: setattr(_hooks, "_hook", h))
        _hooks.get_axon_ntff_profile_hook = lambda: _hooks._hook
        sys.modules["antenv.axon_hooks"] = _hooks
        antenv.axon_hooks = _hooks
    except Exception:  # noqa: BLE001
        pass

F32 = mybir.dt.float32
F16 = mybir.dt.float16
AF = mybir.ActivationFunctionType
OP = mybir.AluOpType

NEG_BIG = -1.0e30


def default_cfg():
    return dict(
        B=2, L=2048, D=4096, QH=4, HD=128,
        CH=512,   # attention q-chunk width
        QN=512,   # qkv projection l-chunk width
        OG=512,   # output projection column-group width
        CLIP=8.0, theta=500000.0, ncores=8,
    )


def mini_cfg():
    return dict(
        B=2, L=512, D=512, QH=2, HD=128,
        CH=256, QN=256, OG=256,
        CLIP=8.0, theta=500000.0, ncores=1,
    )


def build_program(cfg):
    B, L, D = cfg["B"], cfg["L"], cfg["D"]
    QH, HD = cfg["QH"], cfg["HD"]
    CH, QN, OG = cfg["CH"], cfg["QN"], cfg["OG"]
    CLIP = cfg["CLIP"]
    RT = QH + 2               # r-tiles per core: QH q heads, k, v
    R = RT * HD
    CT = D // HD              # contraction tiles
    TPC = CH // HD            # k-tiles per attention chunk
    NJ = L // CH              # attention q-chunks per batch
    NLC = L // QN
    scale = 1.0 / math.sqrt(HD)

    nc = bacc.Bacc("TRN2", target_bir_lowering=False, debug=False,
                   enable_asserts=False, num_devices=1)

    CT2 = CT // 2
    # x and wq are staged host-side in ci-pair-major layout so each DMA
    # start moves a 256KB+ contiguous block (one engine stream each;
    # issue rate on the sync engine is the DMA concurrency limit)
    xH_d = nc.dram_tensor("xH", [B * NLC * CT2 * HD, 2 * QN], F16,
                          kind="ExternalInput").ap()
    wq_d = nc.dram_tensor("wq", [CT2 * HD, 2 * R], F16,
                          kind="ExternalInput").ap()
    wo_d = nc.dram_tensor("wo", [QH * HD, D], F16, kind="ExternalInput").ap()
    ra_d = nc.dram_tensor("ropeA", [HD, L], F16, kind="ExternalInput").ap()
    rb_d = nc.dram_tensor("ropeB", [HD, L], F32, kind="ExternalInput").ap()
    pm_d = nc.dram_tensor("perm", [HD, HD], F16, kind="ExternalInput").ap()
    ms_d = nc.dram_tensor("mask", [HD, HD], F32, kind="ExternalInput").ap()
    ok_d = nc.dram_tensor("onesk", [HD, 1], F16, kind="ExternalInput").ap()
    id_d = nc.dram_tensor("ident", [HD, HD], F16, kind="ExternalInput").ap()
    out_d = nc.dram_tensor("out", [B * L, D], F16, kind="ExternalOutput").ap()

    from contextlib import ExitStack
    with tile.TileContext(nc) as tc, ExitStack() as _es:
        wq_pool = _es.enter_context(tc.tile_pool(name="wq_pool", bufs=1))
        wo_pool = _es.enter_context(tc.tile_pool(name="wo_pool", bufs=1))
        cpool = _es.enter_context(tc.tile_pool(name="const", bufs=1))
        ex_pool = _es.enter_context(tc.tile_pool(name="ex", bufs=9))
        at_pool = _es.enter_context(tc.tile_pool(name="at", bufs=2 * QH + 1))
        rc_pool = _es.enter_context(tc.tile_pool(name="rc", bufs=2))
        bc_pool = _es.enter_context(tc.tile_pool(name="bc", bufs=2))
        fo_pool = _es.enter_context(tc.tile_pool(name="fo", bufs=3))
        qkv_pool = _es.enter_context(tc.tile_pool(name="qkv", bufs=2))
        qp_pool = _es.enter_context(tc.tile_pool(name="qp", bufs=1))
        x_pool = _es.enter_context(tc.tile_pool(name="xp", bufs=20))
        cl_pool = _es.enter_context(tc.tile_pool(name="cl", bufs=1))
        rt_pool = _es.enter_context(tc.tile_pool(name="rt", bufs=2))
        tb_pool = _es.enter_context(tc.tile_pool(name="tb", bufs=2))
        PS = _es.enter_context(tc.tile_pool(name="PS", bufs=2, space="PSUM"))
        PVF = _es.enter_context(tc.tile_pool(name="PVF", bufs=3,
                                             space="PSUM"))
        PM = _es.enter_context(tc.tile_pool(name="PM", bufs=1, space="PSUM"))
        PQ = _es.enter_context(tc.tile_pool(name="PQ", bufs=2, space="PSUM"))
        t_perm = cpool.tile([HD, HD], F16, tag="t_perm")
        t_mask = cpool.tile([HD, HD], F32, tag="t_mask")
        t_onesk = cpool.tile([HD, 1], F16, tag="t_onesk")
        t_ident = cpool.tile([HD, HD], F16, tag="t_ident")
        nc.sync.dma_start(t_perm[:], pm_d[:])
        nc.sync.dma_start(t_mask[:], ms_d[:])
        nc.sync.dma_start(t_onesk[:], ok_d[:])
        nc.sync.dma_start(t_ident[:], id_d[:])

        def emit_x_dmas(b, lc):
            xts = []
            for g in range(CT2):
                xt = x_pool.tile([HD, 2 * QN], F16, tag="xt",
                                 name=f"xt{b}_{lc}_{g}")
                row0 = ((b * NLC + lc) * CT2 + g) * HD
                nc.sync.dma_start(xt[:], xH_d[row0:row0 + HD, :])
                xts.append(xt)
            return xts

        def x_sl(xts, ci):
            return xts[ci // 2][:, (ci % 2) * QN:(ci % 2 + 1) * QN]

        def w_sl(ci, r):
            off = (ci % 2) * R + r * HD
            return w_ci[ci // 2][:, off:off + HD]

        # k,v first so attention on a chunk unblocks one pass earlier
        groups = [(QH, QH + 1)] + [(2 * g, 2 * g + 1) for g in range(QH // 2)]

        # first x chunk interleaved with the qkv weights so the first
        # pass streams both as they land; leading weight tiles split in
        # halves so the first matmuls see their data sooner
        first_xts = []
        w_ci = [wq_pool.tile([HD, 2 * R], F16, tag=f"w{g}", name=f"w{g}")
                for g in range(CT2)]
        for g in range(CT2):
            xt = x_pool.tile([HD, 2 * QN], F16, tag="xt",
                             name=f"xt0_0_{g}")
            nc.sync.dma_start(xt[:], xH_d[g * HD:(g + 1) * HD, :])
            first_xts.append(xt)
            if g < 4:
                nc.sync.dma_start(w_ci[g][:, :R], wq_d[g * HD:(g + 1) * HD, :R])
                nc.sync.dma_start(w_ci[g][:, R:], wq_d[g * HD:(g + 1) * HD, R:])
            else:
                nc.sync.dma_start(w_ci[g][:], wq_d[g * HD:(g + 1) * HD, :])
        # resident output-projection weights (DMAs emitted after the
        # first chunk; only needed ~100us in, at the first emit_fin)
        wo_sb = wo_pool.tile([HD, QH * D], F16, tag="wo_sb", name="wo_sb")

        # q/k/v tiles shared across batches; q is consumed within its own
        # chunk so one slot per tag suffices, k/v need two (cross-batch)
        q_t = [[[qp_pool.tile([HD, QN], F16, tag=f"q{h}_{lc}",
                              name=f"q{h}_{b}_{lc}")
                 for lc in range(NLC)]
                for h in range(QH)]
               for b in range(B)]
        k_t = [[qkv_pool.tile([HD, QN], F16, tag=f"k_{lc}",
                              name=f"k_{b}_{lc}")
                for lc in range(NLC)]
               for b in range(B)]
        v_t = [[qkv_pool.tile([HD, QN], F16, tag=f"v_{lc}",
                              name=f"v_{b}_{lc}")
                for lc in range(NLC)]
               for b in range(B)]

        def emit_rope(b, lc, r, cl, t_ra, t_rb):
            """RoPE for q head r (r<QH) or k (r==QH) from clipped tile."""
            dest = (q_t[b][r][lc] if r < QH else k_t[b][lc])[:]
            rot = PQ.tile([HD, QN], F32, tag="p",
                          name=f"rot{b}_{lc}_{r}")
            nc.tensor.matmul(rot[:], t_perm[:], cl[:],
                             start=True, stop=True)
            t1 = rt_pool.tile([HD, QN], F16, tag="t1")
            nc.vector.tensor_tensor(t1[:], rot[:], t_rb[:], OP.mult)
            nc.vector.tensor_tensor(dest, cl[:], t_ra[:], OP.mult)
            nc.vector.tensor_tensor(dest, dest, t1[:], OP.add)

        def emit_qkv_chunk(b, lc, xts=None):
            t_ra = tb_pool.tile([HD, QN], F16, tag="t_ra",
                                name=f"t_ra{b}_{lc}")
            t_rb = tb_pool.tile([HD, QN], F32, tag="t_rb",
                                name=f"t_rb{b}_{lc}")
            lsl0 = slice(lc * QN, (lc + 1) * QN)
            nc.sync.dma_start(t_ra[:], ra_d[:, lsl0])
            nc.sync.dma_start(t_rb[:], rb_d[:, lsl0])
            if xts is None:
                xts = emit_x_dmas(b, lc)
            # 3 passes of 2 r-tiles over the resident x tiles: holds only
            # the 2 PQ banks so attention keeps its own banks meanwhile.
            for r0, r1 in groups:
                acc = [PQ.tile([HD, QN], F32, tag="p",
                               name=f"acc{b}_{lc}_{r}")
                       for r in (r0, r1)]
                for ci in range(CT):
                    for i, r in enumerate((r0, r1)):
                        nc.tensor.matmul(
                            acc[i][:],
                            w_sl(ci, r),
                            x_sl(xts, ci),
                            start=(ci == 0), stop=(ci == CT - 1))
                for i, r in enumerate((r0, r1)):
                    cl = cl_pool.tile([HD, QN], F16, tag=f"cl{r}",
                                      name=f"cl{b}_{lc}_{r}")
                    nc.vector.tensor_scalar(
                        cl[:], acc[i][:], -CLIP, CLIP, OP.max, OP.min)
                    if r < QH + 1:
                        emit_rope(b, lc, r, cl, t_ra, t_rb)
                    else:
                        vtr = PQ.tile([HD, QN], F16, tag="p",
                                      name=f"vtr{b}_{lc}")
                        nt = QN // HD
                        for t in range(nt):
                            nc.tensor.matmul(
                                vtr[:, t * HD:(t + 1) * HD],
                                cl[:, t * HD:(t + 1) * HD],
                                t_ident[:],
                                is_transpose=True,
                                start=(t == 0), stop=(t == nt - 1))
                        nc.vector.tensor_copy(v_t[b][lc][:], vtr[:])

        def emit_attn_chunk(b, j):
            at_tiles = []
            for h in range(QH):
                pv = PVF.tile([HD, CH], F32, tag="p",
                              name=f"pv{b}_{j}_{h}")
                sm = PM.tile([1, CH], F32, tag="p",
                             name=f"sm{b}_{j}_{h}")
                nk = (j + 1) * TPC
                # sm matmuls are batched: the [1,W] col-group-0 output
                # forces a PE mode switch that slows both neighbours, so
                # emit them in bursts instead of per-ki
                sm_batch = []

                def flush_sm(last):
                    for kb, (w0b, Wb, exb) in enumerate(sm_batch):
                        nc.tensor.matmul(
                            sm[0:1, w0b:w0b + Wb],
                            t_onesk[:], exb[:, :Wb],
                            start=(first_sm[0] and kb == 0),
                            stop=(last and kb == len(sm_batch) - 1))
                    if sm_batch:
                        first_sm[0] = False
                    sm_batch.clear()

                first_sm = [True]
                for ki in range(nk):
                    diag = ki >= j * TPC
                    w0 = (ki - j * TPC) * HD if diag else 0
                    W = CH - w0
                    klc, kof = divmod(ki * HD, QN)
                    sc = PS.tile([HD, CH], F32, tag="p",
                                 name=f"sc{b}_{j}_{h}_{ki}")
                    nc.tensor.matmul(
                        sc[:, :W],
                        k_t[b][klc][:, kof:kof + HD],
                        q_t[b][h][j][:, w0:w0 + W],
                        start=True, stop=True)
                    ex = ex_pool.tile([HD, CH], F16, tag="ex")
                    if diag:
                        nc.vector.tensor_tensor(
                            sc[:, :HD], sc[:, :HD], t_mask[:], OP.add)
                    nc.scalar.activation(
                        ex[:, :W], sc[:, :W], AF.Exp, scale=scale)
                    nc.tensor.matmul(
                        pv[:, w0:w0 + W],
                        v_t[b][klc][:, kof:kof + HD],
                        ex[:, :W],
                        start=(ki == 0), stop=(ki == nk - 1))
                    sm_batch.append((w0, W, ex))
                    if len(sm_batch) == 8 or ki == nk - 1:
                        flush_sm(ki == nk - 1)
                rc32 = rc_pool.tile([1, CH], F32, tag="rc32",
                                    name=f"rc32_{b}_{j}_{h}")
                nc.vector.reciprocal_approx_fast(rc32[:], sm[:])
                bcs = bc_pool.tile([HD, CH], F32, tag="bcs",
                                   name=f"bcs_{b}_{j}_{h}")
                nc.gpsimd.partition_broadcast(bcs[:], rc32[:])
                at = at_pool.tile([HD, CH], F16, tag="at",
                                  name=f"at{b}_{j}_{h}")
                nc.vector.tensor_tensor(at[:], pv[:], bcs[:], OP.mult)
                at_tiles.append(at)
            return at_tiles

        def emit_fin(b, j, at_tiles, tail=False):
            for lt in range(TPC):
                fo = None
                for oc in range(D // OG):
                    fn = PVF.tile([HD, OG], F32, tag="p",
                                  name=f"fn{b}_{j}_{lt}_{oc}")
                    for i in range(QH):
                        nc.tensor.matmul(
                            fn[:],
                            at_tiles[i][:, lt * HD:(lt + 1) * HD],
                            wo_sb[:, i * D + oc * OG:
                                  i * D + (oc + 1) * OG],
                            start=(i == 0), stop=(i == QH - 1))
                    row0 = b * L + j * CH + lt * HD
                    if tail:
                        # smaller parallel stores drain the kernel tail
                        # faster than one long per-engine stream
                        fo = fo_pool.tile([HD, 2 * OG], F16, tag="fo")
                        if oc % 2 == 0:
                            nc.vector.tensor_copy(fo[:, :OG], fn[:])
                        else:
                            nc.scalar.copy(fo[:, :OG], fn[:])
                        nc.sync.dma_start(
                            out_d[row0:row0 + HD,
                                  oc * OG:(oc + 1) * OG],
                            fo[:, :OG])
                        continue
                    # pairs of column groups share one fo tile so the
                    # out DMA moves 256KB per start
                    half = oc % 2
                    if half == 0:
                        fo = fo_pool.tile([HD, 2 * OG], F16, tag="fo")
                        nc.vector.tensor_copy(fo[:, :OG], fn[:])
                    else:
                        nc.scalar.copy(fo[:, OG:], fn[:])
                        nc.sync.dma_start(
                            out_d[row0:row0 + HD,
                                  (oc - 1) * OG:(oc + 1) * OG],
                            fo[:])

        prev = None
        for b in range(B):
            for lc in range(NLC):
                first = (b == 0 and lc == 0)
                emit_qkv_chunk(b, lc, xts=first_xts if first else None)
                at_tiles = emit_attn_chunk(b, j=lc)
                if b == 0 and lc == 1:
                    # after chunk 1's x DMAs so they aren't queued
                    # behind these 4 MB; first use is ~one chunk later
                    for i in range(QH):
                        nc.sync.dma_start(wo_sb[:, i * D:(i + 1) * D],
                                          wo_d[i * HD:(i + 1) * HD, :])
                if prev is not None:
                    emit_fin(prev[0], prev[1], prev[2])
                prev = (b, lc, at_tiles)
        emit_fin(prev[0], prev[1], prev[2], tail=True)
    nc.compile()
    return nc


def host_tables(cfg):
    L, HD, theta = cfg["L"], cfg["HD"], cfg["theta"]
    half = HD // 2
    inv_freq = 1.0 / (theta ** (np.arange(half, dtype=np.float64) / half))
    ang = np.arange(L, dtype=np.float64)[:, None] * inv_freq[None, :]  # [L,half]
    cos = np.cos(ang).astype(np.float32)   # [L, half]
    sin = np.sin(ang).astype(np.float32)
    ropeA = np.empty((HD, L), dtype=np.float32)
    ropeB = np.empty((HD, L), dtype=np.float32)
    ropeA[:half] = cos.T
    ropeA[half:] = cos.T
    ropeB[:half] = sin.T
    ropeB[half:] = sin.T

    perm = np.zeros((HD, HD), dtype=np.float32)
    for d in range(half):
        perm[d + half, d] = -1.0          # rot[d<64] = -q[d+64]
    for d in range(half, HD):
        perm[d - half, d] = 1.0           # rot[d>=64] = +q[d-64]

    mask = np.where(np.arange(HD)[None, :] >= np.arange(HD)[:, None],
                    0.0, NEG_BIG).astype(np.float32)  # [k, q]
    onesk = np.ones((HD, 1), dtype=np.float32)
    ident = np.eye(HD, dtype=np.float32)
    return dict(ropeA=ropeA.astype(np.float16), ropeB=ropeB,
                perm=perm.astype(np.float16), mask=mask,
                onesk=onesk.astype(np.float16),
                ident=ident.astype(np.float16))


def host_x_layout(cfg, x):
    """[B,L,D] -> ci-pair-major rows [(b,lc,g,p), (u,t)] in fp16."""
    B, L, D, HD = cfg["B"], cfg["L"], cfg["D"], cfg["HD"]
    QN = cfg["QN"]
    NLC = L // QN
    CT2 = D // HD // 2
    arr = x.reshape(B, NLC, QN, CT2, 2, HD).transpose(0, 1, 3, 5, 4, 2)
    return np.ascontiguousarray(
        arr.reshape(B * NLC * CT2 * HD, 2 * QN).astype(np.float16))


def host_wq_layout(cfg, wqT):
    """[D,R] -> ci-pair-major rows [(g,p), (u,rcol)] in fp16."""
    D, HD = cfg["D"], cfg["HD"]
    R = wqT.shape[1]
    CT2 = D // HD // 2
    arr = wqT.reshape(CT2, 2, HD, R).transpose(0, 2, 1, 3)
    return np.ascontiguousarray(
        arr.reshape(CT2 * HD, 2 * R).astype(np.float16))


def host_in_maps(cfg, x, Wqkv, Wout):
    """Build per-core input maps from the full tensors."""
    B, L, D, QH, HD = cfg["B"], cfg["L"], cfg["D"], cfg["QH"], cfg["HD"]
    nco = cfg["ncores"]
    tabs = host_tables(cfg)
    xH = host_x_layout(cfg, np.asarray(x))
    NHT = QH * nco      # total q heads
    in_maps = []
    for c in range(nco):
        q_rows = np.arange(c * QH * HD, (c + 1) * QH * HD)
        k_rows = np.arange(NHT * HD + c * HD, NHT * HD + (c + 1) * HD)
        v_rows = np.arange(NHT * HD + nco * HD + c * HD,
                           NHT * HD + nco * HD + (c + 1) * HD)
        rows = np.concatenate([q_rows, k_rows, v_rows])
        wq = host_wq_layout(cfg, Wqkv[rows, :].T)
        cols = np.arange(c * QH * HD, (c + 1) * QH * HD)
        wo = np.ascontiguousarray(Wout[:, cols].T.astype(np.float16))
        m = dict(xH=xH, wq=wq, wo=wo)
        m.update(tabs)
        in_maps.append(m)
    return in_maps


_PROGRAM_CACHE = {}
LAST_RESULTS = None


def _get_program(cfg_key, cfg):
    if cfg_key not in _PROGRAM_CACHE:
        _PROGRAM_CACHE[cfg_key] = build_program(cfg)
    return _PROGRAM_CACHE[cfg_key]


def kernel(x, Wqkv, Wout):
    cfg = default_cfg()
    B, L, D = cfg["B"], cfg["L"], cfg["D"]
    x = np.asarray(x, dtype=np.float32)
    Wqkv = np.asarray(Wqkv, dtype=np.float32)
    Wout = np.asarray(Wout, dtype=np.float32)
    nc = _get_program("full", cfg)
    in_maps = host_in_maps(cfg, x, Wqkv, Wout)
    res = bass_utils.run_bass_kernel_spmd(
        nc, in_maps, core_ids=list(range(cfg["ncores"])))
    global LAST_RESULTS
    LAST_RESULTS = res
    acc = np.zeros((B * L, D), dtype=np.float32)
    for c in range(cfg["ncores"]):
        acc += res.results[c]["out"].astype(np.float32)
    return acc.reshape(B, L, D)


# ---------------------------------------------------------------------------
# dev helpers (not used by the grading harness)

def _np_partial_reference(cfg, x, Wqkv_rows, Wout_cols_T):
    """Numpy reference for ONE core's partial output.

    Wqkv_rows: [R, D] (q heads, k, v rows for this core)
    Wout_cols_T: [QH*HD, D] (transposed slice of Wout columns)
    """
    B, L, D, QH, HD = cfg["B"], cfg["L"], cfg["D"], cfg["QH"], cfg["HD"]
    CLIP, theta = cfg["CLIP"], cfg["theta"]
    half = HD // 2
    xf = x.reshape(B * L, D).astype(np.float64)
    qkv = np.clip(xf @ Wqkv_rows.astype(np.float64).T, -CLIP, CLIP)
    qkv = qkv.reshape(B, L, (QH + 2), HD)
    q = qkv[:, :, :QH, :]            # [B, L, QH, HD]
    k = qkv[:, :, QH, :]             # [B, L, HD]
    v = qkv[:, :, QH + 1, :]         # [B, L, HD]

    inv_freq = 1.0 / (theta ** (np.arange(half, dtype=np.float64) / half))
    ang = np.arange(L, dtype=np.float64)[:, None] * inv_freq[None, :]
    cos, sin = np.cos(ang), np.sin(ang)      # [L, half]

    def rope(t):  # t [B, L, ..., HD] with positions on axis 1
        t1, t2 = t[..., :half], t[..., half:]
        shape = [1, L] + [1] * (t.ndim - 3) + [half]
        c = cos.reshape(L, half).reshape(shape)
        s = sin.reshape(L, half).reshape(shape)
        return np.concatenate([t1 * c - t2 * s, t2 * c + t1 * s], axis=-1)

    q = rope(q)
    k = rope(k)
    scalev = 1.0 / math.sqrt(HD)
    causal = np.tril(np.ones((L, L), dtype=bool))
    outs = []
    for bi in range(B):
        heads = []
        for h in range(QH):
            s = (q[bi, :, h, :] @ k[bi].T) * scalev
            s = np.where(causal, s, -np.inf)
            p = np.exp(s - s.max(axis=-1, keepdims=True))
            p /= p.sum(axis=-1, keepdims=True)
            heads.append(p @ v[bi])
        attn = np.concatenate(heads, axis=-1)     # [L, QH*HD]
        outs.append(attn)
    attn = np.stack(outs, 0).reshape(B * L, QH * HD)
    return (attn @ Wout_cols_T.astype(np.float64)).astype(np.float32)


def _mini_test(mode="sim"):
    from concourse.bass_interp import CoreSim
    cfg = mini_cfg()
    B, L, D, QH, HD = cfg["B"], cfg["L"], cfg["D"], cfg["QH"], cfg["HD"]
    R = (QH + 2) * HD
    rng = np.random.default_rng(0)
    x = (rng.standard_normal((B, L, D)) * 1.0).astype(np.float32)
    Wqkv_rows = (rng.standard_normal((R, D)) * D ** -0.5).astype(np.float32)
    WoT = (rng.standard_normal((QH * HD, D)) * D ** -0.5).astype(np.float32)

    nc = build_program(cfg)
    tabs = host_tables(cfg)
    xH = host_x_layout(cfg, x)
    wq = host_wq_layout(cfg, Wqkv_rows.T.astype(np.float64))
    in_map = dict(xH=xH, wq=wq, wo=WoT.astype(np.float16))
    in_map.update(tabs)

    want = _np_partial_reference(cfg, x, Wqkv_rows, WoT)

    if mode == "sim":
        sim = CoreSim(nc, trace=False)
        for kk, vv in in_map.items():
            sim.tensor(kk)[:] = vv
        sim.simulate(check_with_hw=False)
        got = np.array(sim.tensor("out")).astype(np.float32)
    else:
        res = bass_utils.run_bass_kernel_spmd(nc, [in_map], core_ids=[0])
        got = res.results[0]["out"].astype(np.float32)
    relmax = np.abs(got - want).max() / np.abs(want).max()
    rel2 = np.linalg.norm(got - want) / np.linalg.norm(want)
    print(f"mini {mode}: relmax={relmax:.3e} rel2={rel2:.3e}")


if __name__ == "__main__":
    _mini_test(sys.argv[1] if len(sys.argv) > 1 else "sim")
